# revision 4
# baseline (speedup 1.0000x reference)
"""Distributed multi-head attention for TRN2 (8 NeuronCores).

Reference computation (per batch b):
    qkv = x @ w_qkv.T                         # (N, 3C)
    q, k, v = split/reshape to (H, N, D)
    attn = softmax(q @ k.T * D**-0.5)         # per head
    out = (attn @ v) reassembled to (N, C)
    out = out @ w_proj.T + b_proj

Sharding: 8 cores = 4 batches x 2 query-halves. Each core computes k/v
for all 2048 tokens of its batch (duplicated across the 2 cores of a
batch - cheaper than communicating), q for its own 1024 tokens, the
full attention for all 12 heads over its 1024 queries, and the output
projection. No collectives.

Layout strategy (all chosen so no on-chip transposes are needed):
  - host passes x^T and w_qkv^T so projections contract over partitions
  - q,k are produced "d-major" ([head-dim, tokens]) via out^T-form
    matmuls; scores are computed transposed ([keys, queries]) which is
    exactly the layout attn@v consumes as its stationary-side operand
  - softmax needs no max-subtraction (scores ~ N(0,1), fp32 exp range)
  - the denominator rides along as a ones-column appended to v (M=65
    matmuls); normalization uses a K=1 ones-matmul to broadcast 1/denom
    across partitions
  - all matmuls in bf16 (PSUM accumulation is fp32); softmax exp runs
    on the scalar (ACT) engine from PSUM f32, writing bf16 probs

Schedule: one flat software pipeline over 96 score-steps (12 passes x
8 k-block pairs), passes ordered qc-major. Scores for step t+1 are
emitted before step t's attn@v so the ACT engine (the per-step floor,
~2.1us/step) never waits at pass boundaries. Everything else -- the
qkv projections, and the output projection for the first query half --
rides in deadline-scheduled filler slots; out-proj qc=0 fills the
otherwise ACT-bound back half of the pipeline. The pre-phase is just 2
projection units, and the startup DMAs are chunked per 128x512 tile so
the first matmul fires as soon as its first chunk lands.

Self-contained: hardcodes B=4, N=2048, C=768, H=12, D=64.
"""

import numpy as np
import ml_dtypes

import concourse.bass as bass
import concourse.mybir as mybir
from concourse import bacc
from concourse.tile import TileContext
from concourse.bass_utils import run_bass_kernel_spmd

F32 = mybir.dt.float32
BF16 = mybir.dt.bfloat16
EXP = mybir.ActivationFunctionType.Exp

B, N, C = 4, 2048, 768
H, D = 12, 64
SCALE = float(D) ** -0.5  # 0.125
NQ = N // 2  # queries per core: 1024
CB = C // 128  # 6 c-chunks
TB = N // 128  # 16 token blocks
HB = H // 2  # 6 head pairs
VW = H * (D + 1)  # 780: v block width with ones columns

N_CORES = 8

# w_qkv columns, grouped in the order the projection units consume them:
# pair-0 k/q, v (split 512+256 for finer DMA deps), then k/q for pairs
# 1..5. Each group holds its column range for all six 128-row input
# chunks, contiguously.
_WQ_GROUPS = [(C, 128), (0, 128), (2 * C, 512), (2 * C + 512, 256)]
for _ob in range(1, CB):
    _WQ_GROUPS.append((C + _ob * 128, 128))
    _WQ_GROUPS.append((_ob * 128, 128))
_WQ_BASE = {}
_cur = 0
for _o0, _w in _WQ_GROUPS:
    _WQ_BASE[_o0] = (_cur, _w)
    _cur += CB * _w

# pass order: qc-major so attnT for qc=0 is complete by mid-pipeline and
# the qc=0 output projection can fill the ACT-bound later passes
PASSES = [(hb, 0) for hb in range(HB)] + [(hb, 1) for hb in range(HB)]
NSTEPS = len(PASSES) * 8  # 96 score-steps, 2 k-blocks each


def _build():
    nc = bacc.Bacc(None, target_bir_lowering=False)

    # host-packed SBUF images: xTp cols = [tch][ci][t]; wqp cols grouped
    # in consumption order (see _WQ_GROUPS)
    xTp = nc.declare_dram_parameter("xTp", [128, CB * N], BF16, isOutput=False)
    wqp = nc.declare_dram_parameter("wqp", [128, CB * 3 * C], BF16, isOutput=False)
    wprojp = nc.declare_dram_parameter("wprojp", [128, CB * C], BF16, isOutput=False)
    biasp = nc.declare_dram_parameter("biasp", [128, CB], F32, isOutput=False)
    outT = nc.declare_dram_parameter("outT", [C, NQ], BF16, isOutput=True)

    with TileContext(nc) as tc:
        with (
            tc.tile_pool(name="per", bufs=1) as per,
            tc.tile_pool(name="p23", bufs=1) as p23,
            tc.tile_pool(name="hp", bufs=8) as hp,
            tc.tile_pool(name="mi", bufs=3) as mi,
            tc.tile_pool(name="op", bufs=2) as op_pool,
            tc.tile_pool(name="wq", bufs=1) as wq_pool,
            tc.tile_pool(name="xt", bufs=1) as xt_pool,
            tc.tile_pool(name="ps", bufs=2, space="PSUM") as ps2,
        ):
            # ---- persistent tiles -------------------------------------
            qT_sb = per.tile([128, CB * NQ], BF16)  # q^T  [2 heads/blk, 1024]
            kT_sb = per.tile([128, CB * N], BF16)  # k^T  [2 heads/blk, 2048]
            vaug_sb = per.tile([128, TB * VW], BF16)  # v + ones cols
            bias_sb = per.tile([128, CB], F32)
            ones_sb = per.tile([1, 64], BF16)
            attnT_sb = p23.tile([128, CB * NQ], BF16)  # attn out^T
            wproj_sb = p23.tile([128, CB * C], BF16)

            nc.vector.memset(ones_sb[:, :], 1.0)
            # ones columns of vaug: col 64 of each 65-wide head slot
            vaug_ones = vaug_sb[:, :].rearrange(
                "p (t h x) -> p t h x", t=TB, h=H, x=D + 1
            )[:, :, :, D : D + 1]
            nc.vector.memset(vaug_ones, 1.0)

            wqkv_sb = wq_pool.tile([128, CB * 3 * C], BF16)
            # x chunks as separate [128, 512] tiles per (tch, ci) so
            # startup DMA deps are fine-grained
            xts = [
                [
                    xt_pool.tile([128, 512], BF16, name=f"xt{t}_{ci}")
                    for ci in range(CB)
                ]
                for t in range(4)
            ]

            def _dma_xt_ci(tch, ci):
                base = tch * CB * 512 + ci * 512
                nc.sync.dma_start(
                    out=xts[tch][ci][:, :], in_=xTp[:, base : base + 512]
                )

            def _dma_wq(gi, ci=None):
                o0, w = _WQ_GROUPS[gi]
                base, _ = _WQ_BASE[o0]
                if ci is None:
                    nc.sync.dma_start(
                        out=wqkv_sb[:, base : base + CB * w],
                        in_=wqp[:, base : base + CB * w],
                    )
                else:
                    b0 = base + ci * w
                    nc.sync.dma_start(
                        out=wqkv_sb[:, b0 : b0 + w], in_=wqp[:, b0 : b0 + w]
                    )

            # DMA priority order (first-consumption order):
            #   pair0-k & x-chunk0 (pre-phase), pair0-q, x-chunk1 (k filler),
            #   v weights, x-chunks 2-3, later pairs' k/q, bias/wproj
            for ci in range(CB):
                _dma_wq(0, ci)
                _dma_xt_ci(0, ci)
            for ci in range(CB):
                _dma_wq(1, ci)
            for ci in range(CB):
                _dma_xt_ci(1, ci)
            for ci in range(CB):
                _dma_wq(2, ci)
            for ci in range(CB):
                _dma_wq(3, ci)
            for ci in range(CB):
                _dma_xt_ci(2, ci)
            for ci in range(CB):
                _dma_xt_ci(3, ci)
            for gi in range(4, len(_WQ_GROUPS)):
                _dma_wq(gi)
            nc.sync.dma_start(out=bias_sb[:, :], in_=biasp[:, :])
            nc.sync.dma_start(out=wproj_sb[:, :], in_=wprojp[:, :])

            def wq(ci, o0, width):
                base, gw = _WQ_BASE[o0]
                return wqkv_sb[:, base + ci * gw : base + ci * gw + width]

            # ---- projection work units (PE filler) --------------------
            def kq_unit(ob, tch, is_q):
                """one k^T (or q^T) block: out-dims block ob, 512 tokens"""
                t0 = tch * 512
                kind = "q" if is_q else "k"
                psv = ps2.tile(
                    [128, 512], F32, tag="psV", bufs=2, name=f"{kind}{ob}_{tch}"
                )
                for ci in range(CB):
                    nc.tensor.matmul(
                        psv[:, :],
                        wq(ci, (0 if is_q else C) + ob * 128, 128),
                        xts[tch][ci][:, :],
                        start=(ci == 0),
                        stop=(ci == CB - 1),
                    )
                if is_q:
                    nc.vector.tensor_copy(
                        qT_sb[:, ob * NQ + t0 : ob * NQ + t0 + 512], psv[:, :]
                    )
                else:
                    nc.vector.tensor_copy(
                        kT_sb[:, ob * N + t0 : ob * N + t0 + 512], psv[:, :]
                    )

            def v_unit(t128, o0, w):
                """one v unit: 128 tokens x [o0, o0+w) v-dims, written
                (bf16) into the vaug slot layout"""
                tch, tb = divmod(t128, 4)
                psv = ps2.tile(
                    [128, 512], F32, tag="psV", bufs=2, name=f"v{t128}_{o0}"
                )
                for ci in range(CB):
                    nc.tensor.matmul(
                        psv[:, :w],
                        xts[tch][ci][:, tb * 128 : (tb + 1) * 128],
                        wq(ci, 2 * C + o0, w),
                        start=(ci == 0),
                        stop=(ci == CB - 1),
                    )
                nh = w // D
                src = psv[:, :w].rearrange("p (h x) -> p h x", x=D)
                h0 = o0 // D
                base = t128 * VW + h0 * (D + 1)
                dst = vaug_sb[:, base : base + nh * (D + 1)].rearrange(
                    "p (h x) -> p h x", x=D + 1
                )[:, :, :D]
                nc.vector.tensor_copy(dst, src)

            def proj_unit(ob, qc):
                """output projection for out-dims block ob, query half qc
                (contracts all 6 attnT pair-blocks), bias add + DMA out"""
                psp = ps2.tile(
                    [128, 512], F32, tag="psV", bufs=2, name=f"prj{ob}_{qc}"
                )
                for cb in range(CB):
                    nc.tensor.matmul(
                        psp[:, :],
                        wproj_sb[:, cb * C + ob * 128 : cb * C + (ob + 1) * 128],
                        attnT_sb[:, cb * NQ + qc * 512 : cb * NQ + (qc + 1) * 512],
                        start=(cb == 0),
                        stop=(cb == CB - 1),
                    )
                ot = op_pool.tile([128, 512], BF16, tag="out")
                nc.vector.tensor_scalar_add(
                    ot[:, :], psp[:, :], bias_sb[:, ob : ob + 1]
                )
                nc.sync.dma_start(
                    out=outT[ob * 128 : (ob + 1) * 128, qc * 512 : (qc + 1) * 512],
                    in_=ot[:, :],
                )

            # ---- filler schedule --------------------------------------
            # filler[t] = list of closures to emit in step t's filler slot
            filler = [[] for _ in range(NSTEPS)]

            def _sched(t, fn):
                filler[t].append(fn)

            # v blocks just-in-time for pass 0: block kb in step kb//2
            for kb in range(TB):
                _sched(kb // 2, (lambda kb=kb: v_unit(kb, 0, 512)))
                _sched(kb // 2, (lambda kb=kb: v_unit(kb, 512, 256)))
            # remaining pair-0 k chunks: k tch needed by scores kb=4*tch
            # (step 2*tch)
            for tch in range(1, 4):
                _sched(2 * (tch - 1), (lambda t=tch: kq_unit(0, t, False)))
            # pairs 1-5: k tch j needed by step 8*hb + 2*j, q tch0 by 8*hb
            for hb in range(1, HB):
                s0 = 8 * (hb - 1) + 3
                for j in range(4):
                    _sched(min(s0 + j, 8 * hb + 2 * j - 2),
                           (lambda h=hb, j=j: kq_unit(h, j, False)))
                _sched(8 * hb - 2, (lambda h=hb: kq_unit(h, 0, True)))
            # q tch1 units needed by pass (hb, 1) = step 48 + 8*hb
            for hb in range(HB):
                _sched(40 + hb, (lambda h=hb: kq_unit(h, 1, True)))
            # out-proj qc=0: attnT qc0 final after pass idx 5 epilogue
            # (multiply lands ~step 50); spread over the ACT-bound tail
            for ob in range(CB):
                _sched(54 + 4 * ob, (lambda o=ob: proj_unit(o, 0)))

            # ---- attention pipeline -----------------------------------
            def epi_pe(hb_, qc_, outs_):
                """PE part of a pass's normalization epilogue. The two
                heads' 1/denom broadcasts go to different column strips of
                one PSUM tile (col tiling) so they run concurrently."""
                psb = ps2.tile(
                    [128, 512], F32, tag="psV", bufs=2,
                    name=f"psb{hb_}_{qc_}",
                )
                for hh_ in range(2):
                    nc.tensor.matmul(
                        psb[64 * hh_ : 64 * hh_ + 64, :],
                        ones_sb[:, :],
                        outs_[hh_][1][:, :],
                        start=True,
                        stop=True,
                    )
                for hh_ in range(2):
                    nc.vector.tensor_mul(
                        attnT_sb[
                            64 * hh_ : 64 * hh_ + 64,
                            hb_ * NQ + qc_ * 512 : hb_ * NQ + (qc_ + 1) * 512,
                        ],
                        psb[64 * hh_ : 64 * hh_ + 64, :],
                        outs_[hh_][0][:, :],
                    )

            def emit_scores(hb, qc, kb2):
                """scores for k-blocks kb2, kb2+1 (both heads) -> psS
                tiles + exp -> probs; returns [(kb, probs), ...]"""
                q0 = hb * NQ + qc * 512
                out = []
                for kb in (kb2, kb2 + 1):
                    sc = ps2.tile(
                        [128, NQ], F32, tag="psS", bufs=2,
                        name=f"sc{hb}_{qc}_{kb}",
                    )
                    for hh in range(2):
                        p0 = 64 * hh
                        nc.tensor.matmul(
                            sc[:, hh * 512 : (hh + 1) * 512],
                            kT_sb[
                                p0 : p0 + 64,
                                hb * N + kb * 128 : hb * N + (kb + 1) * 128,
                            ],
                            qT_sb[p0 : p0 + 64, q0 : q0 + 512],
                            start=True,
                            stop=True,
                            tile_position=(p0, 0),
                        )
                    out.append((kb, sc))
                return out

            def emit_exps(pending_sc):
                out = []
                for kb, sc in pending_sc:
                    pb = hp.tile([128, NQ], BF16, tag="probs")
                    nc.scalar.activation(pb[:, :], sc[:, :], EXP, scale=SCALE)
                    out.append((kb, pb))
                return out

            def av_mms(accs, hb, pkb, ppb):
                for hh in range(2):
                    vs = pkb * VW + (2 * hb + hh) * (D + 1)
                    nc.tensor.matmul(
                        accs[hh][0:65, :],
                        vaug_sb[:, vs : vs + D + 1],
                        ppb[:, hh * 512 : (hh + 1) * 512],
                        start=(pkb == 0),
                        stop=(pkb == TB - 1),
                    )

            def drain(accs):
                """epilogue DVE part: drain accumulators + 1/denominator"""
                outs = []
                for hh in range(2):
                    acc = accs[hh]
                    cpy = mi.tile([64, 512], F32, tag="cpy")
                    nc.vector.tensor_copy(cpy[:, :], acc[0:64, :])
                    den = mi.tile([1, 512], F32, tag="den")
                    nc.vector.tensor_copy(den[:, :], acc[64:65, :])
                    rec = mi.tile([1, 512], F32, tag="rec")
                    nc.vector.reciprocal_approx_fast(rec[:, :], den[:, :])
                    row = mi.tile([1, 512], BF16, tag="row")
                    nc.vector.tensor_copy(row[:, :], rec[:, :])
                    outs.append((cpy, row))
                return outs

            # ---- pre-phase: minimal (pair-0 k/q for the first chunk) --
            kq_unit(0, 0, False)
            kq_unit(0, 0, True)

            # ---- the flat pipeline ------------------------------------
            accs = None  # current pass's psA accumulators
            prev_probs = []  # [(kb, probs)] awaiting attn@v
            prev_pass = None  # (hb, qc) owning prev_probs
            pend_epi = None  # (hb, qc, outs, due_step)

            def new_accs(hb, qc):
                return [
                    ps2.tile(
                        [128, 512], F32, tag="psA", bufs=2,
                        name=f"acc{hb}_{qc}_{i}",
                    )
                    for i in range(2)
                ]

            for t in range(NSTEPS):
                hb, qc = PASSES[t // 8]
                kb2 = (t % 8) * 2
                # scores first: keeps ACT fed across pass boundaries
                pend_sc = emit_scores(hb, qc, kb2)
                # attn@v for the previous step's probs
                for pkb, ppb in prev_probs:
                    av_mms(accs, prev_pass[0], pkb, ppb)
                if prev_probs and prev_probs[-1][0] == TB - 1:
                    # previous pass complete: drain + defer its epilogue,
                    # then recycle the acc slots for the new pass
                    outs = drain(accs)
                    pend_epi = (prev_pass[0], prev_pass[1], outs, t + 1)
                    accs = new_accs(hb, qc)
                elif accs is None:
                    accs = new_accs(hb, qc)
                # filler work
                for fn in filler[t]:
                    fn()
                # deferred epilogue of the pass before last
                if pend_epi is not None and t >= pend_epi[3]:
                    epi_pe(pend_epi[0], pend_epi[1], pend_epi[2])
                    pend_epi = None
                # exp of this step's scores
                prev_probs = emit_exps(pend_sc)
                prev_pass = (hb, qc)

            # ---- tail: last pass's attn@v drain + epilogue ------------
            for pkb, ppb in prev_probs:
                av_mms(accs, prev_pass[0], pkb, ppb)
            outs = drain(accs)
            if pend_epi is not None:
                epi_pe(pend_epi[0], pend_epi[1], pend_epi[2])
            epi_pe(prev_pass[0], prev_pass[1], outs)

            # ---- output projection, qc=1 ------------------------------
            for ob in range(CB):
                proj_unit(ob, 1)

    nc.finalize()
    return nc


_NC_CACHE = []


def _get_nc():
    if not _NC_CACHE:
        _NC_CACHE.append(_build())
    return _NC_CACHE[0]


def kernel(x, w_qkv, w_proj, b_proj):
    x = np.asarray(x, dtype=np.float32)
    w_qkv = np.asarray(w_qkv, dtype=np.float32)
    w_proj = np.asarray(w_proj, dtype=np.float32)
    b_proj = np.asarray(b_proj, dtype=np.float32)

    nc = _get_nc()

    wqkvT = w_qkv.T.astype(ml_dtypes.bfloat16)  # [C, 3C]
    wq3 = np.ascontiguousarray(wqkvT).reshape(CB, 128, 3 * C)  # [ci, p, o]
    wqp = np.concatenate(
        [
            wq3[:, :, o0 : o0 + w].transpose(1, 0, 2).reshape(128, CB * w)
            for o0, w in _WQ_GROUPS
        ],
        axis=1,
    )
    wqp = np.ascontiguousarray(wqp)
    # SBUF images: wproj cols = [ci][o], bias cols = [ci]
    wprojp = np.ascontiguousarray(
        w_proj.T.astype(ml_dtypes.bfloat16).reshape(CB, 128, C)
        .transpose(1, 0, 2)
        .reshape(128, CB * C)
    )
    biasp = np.ascontiguousarray(
        b_proj.astype(np.float32).reshape(CB, 128).T
    )

    in_maps = []
    for core in range(N_CORES):
        b, half = divmod(core, 2)
        # own 1024 query tokens first, then the other half (key order
        # within attention is permutation-invariant)
        mine = x[b, half * NQ : (half + 1) * NQ].T
        other = x[b, (1 - half) * NQ : (2 - half) * NQ].T
        xTc = np.concatenate([mine, other], axis=1).astype(ml_dtypes.bfloat16)
        # pack to the SBUF image: cols = [tch][ci][t]
        xTp = np.ascontiguousarray(
            xTc.reshape(CB, 128, 4, 512).transpose(1, 2, 0, 3).reshape(128, CB * N)
        )
        in_maps.append({"xTp": xTp, "wqp": wqp, "wprojp": wprojp, "biasp": biasp})

    res = run_bass_kernel_spmd(nc, in_maps, core_ids=list(range(N_CORES)))

    out = np.empty((B, N, C), dtype=np.float32)
    for core in range(N_CORES):
        b, half = divmod(core, 2)
        out[b, half * NQ : (half + 1) * NQ, :] = (
            res.results[core]["outT"].astype(np.float32).T
        )
    return out


# revision 12
# speedup vs baseline: 1.1918x; 1.1918x over previous
"""Distributed multi-head attention for TRN2 (8 NeuronCores).

Reference computation (per batch b):
    qkv = x @ w_qkv.T                         # (N, 3C)
    q, k, v = split/reshape to (H, N, D)
    attn = softmax(q @ k.T * D**-0.5)         # per head
    out = (attn @ v) reassembled to (N, C)
    out = out @ w_proj.T + b_proj

Sharding: 8 cores = 4 batches x 2 query-halves. Each core computes k/v
for all 2048 tokens of its batch (duplicated across the 2 cores of a
batch - cheaper than communicating), q for its own 1024 tokens, the
full attention for all 12 heads over its 1024 queries, and the output
projection. No collectives.

Layout strategy (all chosen so no on-chip transposes are needed):
  - host passes x^T and w_qkv^T so projections contract over partitions
  - q,k are produced "d-major" ([head-dim, tokens]) via out^T-form
    matmuls; scores are computed transposed ([keys, queries]) which is
    exactly the layout attn@v consumes as its stationary-side operand
  - softmax needs no max-subtraction (scores ~ N(0,1), fp32 exp range)
  - the denominator rides along as a ones-column appended to v (M=65
    matmuls); normalization uses a K=1 ones-matmul to broadcast 1/denom
    across partitions
  - all matmuls in bf16 (PSUM accumulation is fp32); softmax exp runs
    on the scalar (ACT) engine from PSUM f32, writing bf16 probs

Schedule: one flat software pipeline over 96 score-steps (12 passes x
8 k-block pairs), passes ordered qc-major. Scores for step t+1 are
emitted before step t's attn@v so the ACT engine (the per-step floor,
~2.1us/step) never waits at pass boundaries. Everything else -- the
qkv projections, and the output projection for the first query half --
rides in deadline-scheduled filler slots; out-proj qc=0 fills the
otherwise ACT-bound back half of the pipeline. The pre-phase is just 2
projection units, and the startup DMAs are chunked per 128x512 tile so
the first matmul fires as soon as its first chunk lands.

Self-contained: hardcodes B=4, N=2048, C=768, H=12, D=64.
"""

import numpy as np
import ml_dtypes

import concourse.bass as bass
import concourse.mybir as mybir
from concourse import bacc
from concourse.tile import TileContext
from concourse.bass_utils import run_bass_kernel_spmd

F32 = mybir.dt.float32
BF16 = mybir.dt.bfloat16
EXP = mybir.ActivationFunctionType.Exp

B, N, C = 4, 2048, 768
H, D = 12, 64
SCALE = float(D) ** -0.5  # 0.125
NQ = N // 2  # queries per core: 1024
CB = C // 128  # 6 c-chunks
TB = N // 128  # 16 token blocks
HB = H // 2  # 6 head pairs
VW = H * (D + 1)  # 780: v block width with ones columns

N_CORES = 8

# w_qkv columns, grouped in the order the projection units consume them:
# pair-0 k/q, v (split 512+256 for finer DMA deps), then k/q for pairs
# 1..5. Each group holds its column range for all six 128-row input
# chunks, contiguously.
_WQ_GROUPS = [(C, 128), (0, 128), (2 * C, 512), (2 * C + 512, 256)]
for _ob in range(1, CB):
    _WQ_GROUPS.append((C + _ob * 128, 128))
    _WQ_GROUPS.append((_ob * 128, 128))
_WQ_BASE = {}
_cur = 0
for _o0, _w in _WQ_GROUPS:
    _WQ_BASE[_o0] = (_cur, _w)
    _cur += CB * _w

# pass order: qc-major so attnT for qc=0 is complete by mid-pipeline and
# the qc=0 output projection can fill the ACT-bound later passes
PASSES = [(hb, 0) for hb in range(HB)] + [(hb, 1) for hb in range(HB)]
NSTEPS = len(PASSES) * 8  # 96 score-steps, 2 k-blocks each


def _build():
    nc = bacc.Bacc(None, target_bir_lowering=False)

    # host-packed SBUF images: xTp cols = [tch][ci][t]; wqp cols grouped
    # in consumption order (see _WQ_GROUPS)
    xTp = nc.declare_dram_parameter("xTp", [128, CB * N], BF16, isOutput=False)
    wqp = nc.declare_dram_parameter("wqp", [128, CB * 3 * C], BF16, isOutput=False)
    wprojp = nc.declare_dram_parameter("wprojp", [128, CB * C], BF16, isOutput=False)
    biasp = nc.declare_dram_parameter("biasp", [128, CB], F32, isOutput=False)
    outT = nc.declare_dram_parameter("outT", [C, NQ], BF16, isOutput=True)

    with TileContext(nc) as tc:
        with (
            tc.tile_pool(name="per", bufs=1) as per,
            tc.tile_pool(name="p23", bufs=1) as p23,
            tc.tile_pool(name="hp", bufs=8) as hp,
            tc.tile_pool(name="mi", bufs=3) as mi,
            tc.tile_pool(name="op", bufs=2) as op_pool,
            tc.tile_pool(name="wq", bufs=1) as wq_pool,
            tc.tile_pool(name="xt", bufs=1) as xt_pool,
            tc.tile_pool(name="ps", bufs=2, space="PSUM") as ps2,
        ):
            # ---- persistent tiles -------------------------------------
            qT_sb = per.tile([128, CB * NQ], BF16)  # q^T  [2 heads/blk, 1024]
            kT_sb = per.tile([128, CB * N], BF16)  # k^T  [2 heads/blk, 2048]
            vaug_sb = per.tile([128, TB * VW], BF16)  # v + ones cols
            bias_sb = per.tile([128, CB], F32)
            ones_sb = per.tile([1, 64], BF16)
            attnT_sb = p23.tile([128, CB * NQ], BF16)  # attn out^T
            wproj_sb = p23.tile([128, CB * C], BF16)

            nc.vector.memset(ones_sb[:, :], 1.0)
            # ones columns of vaug: col 64 of each 65-wide head slot
            vaug_ones = vaug_sb[:, :].rearrange(
                "p (t h x) -> p t h x", t=TB, h=H, x=D + 1
            )[:, :, :, D : D + 1]
            nc.vector.memset(vaug_ones, 1.0)

            wqkv_sb = wq_pool.tile([128, CB * 3 * C], BF16)
            # x chunks as separate [128, 512] tiles per (tch, ci) so
            # startup DMA deps are fine-grained
            xts = [
                [
                    xt_pool.tile([128, 512], BF16, name=f"xt{t}_{ci}")
                    for ci in range(CB)
                ]
                for t in range(4)
            ]

            def _dma_xt_ci(tch, ci):
                base = tch * CB * 512 + ci * 512
                nc.sync.dma_start(
                    out=xts[tch][ci][:, :], in_=xTp[:, base : base + 512]
                )

            def _dma_wq(gi, ci=None):
                o0, w = _WQ_GROUPS[gi]
                base, _ = _WQ_BASE[o0]
                if ci is None:
                    nc.sync.dma_start(
                        out=wqkv_sb[:, base : base + CB * w],
                        in_=wqp[:, base : base + CB * w],
                    )
                else:
                    b0 = base + ci * w
                    nc.sync.dma_start(
                        out=wqkv_sb[:, b0 : b0 + w], in_=wqp[:, b0 : b0 + w]
                    )

            # DMA priority order (first-consumption order):
            #   pair0-k & x-chunk0 (pre-phase), pair0-q, x-chunk1 (k filler),
            #   v weights, x-chunks 2-3, later pairs' k/q, bias/wproj
            for ci in range(CB):
                _dma_wq(0, ci)
                _dma_xt_ci(0, ci)
            for ci in range(CB):
                _dma_wq(1, ci)
            for ci in range(CB):
                _dma_xt_ci(1, ci)
            for ci in range(CB):
                _dma_wq(2, ci)
            for ci in range(CB):
                _dma_wq(3, ci)
            for ci in range(CB):
                _dma_xt_ci(2, ci)
            for ci in range(CB):
                _dma_xt_ci(3, ci)
            for gi in range(4, len(_WQ_GROUPS)):
                _dma_wq(gi)
            nc.sync.dma_start(out=bias_sb[:, :], in_=biasp[:, :])
            nc.sync.dma_start(out=wproj_sb[:, :], in_=wprojp[:, :])

            def wq(ci, o0, width):
                base, gw = _WQ_BASE[o0]
                return wqkv_sb[:, base + ci * gw : base + ci * gw + width]

            # ---- projection work units (PE filler) --------------------
            def kq_unit(ob, tch, is_q):
                """one k^T (or q^T) block: out-dims block ob, 512 tokens"""
                t0 = tch * 512
                kind = "q" if is_q else "k"
                psv = ps2.tile(
                    [128, 512], F32, tag="psV", bufs=2, name=f"{kind}{ob}_{tch}"
                )
                for ci in range(CB):
                    nc.tensor.matmul(
                        psv[:, :],
                        wq(ci, (0 if is_q else C) + ob * 128, 128),
                        xts[tch][ci][:, :],
                        start=(ci == 0),
                        stop=(ci == CB - 1),
                    )
                if is_q:
                    nc.vector.tensor_copy(
                        qT_sb[:, ob * NQ + t0 : ob * NQ + t0 + 512], psv[:, :]
                    )
                else:
                    nc.vector.tensor_copy(
                        kT_sb[:, ob * N + t0 : ob * N + t0 + 512], psv[:, :]
                    )

            def v_unit(t128, o0, w):
                """one v unit: 128 tokens x [o0, o0+w) v-dims, written
                (bf16) into the vaug slot layout"""
                tch, tb = divmod(t128, 4)
                psv = ps2.tile(
                    [128, 512], F32, tag="psV", bufs=2, name=f"v{t128}_{o0}"
                )
                for ci in range(CB):
                    nc.tensor.matmul(
                        psv[:, :w],
                        xts[tch][ci][:, tb * 128 : (tb + 1) * 128],
                        wq(ci, 2 * C + o0, w),
                        start=(ci == 0),
                        stop=(ci == CB - 1),
                    )
                nh = w // D
                src = psv[:, :w].rearrange("p (h x) -> p h x", x=D)
                h0 = o0 // D
                base = t128 * VW + h0 * (D + 1)
                dst = vaug_sb[:, base : base + nh * (D + 1)].rearrange(
                    "p (h x) -> p h x", x=D + 1
                )[:, :, :D]
                nc.vector.tensor_copy(dst, src)

            def proj_unit(ob, qc):
                """output projection for out-dims block ob, query half qc
                (contracts all 6 attnT pair-blocks), bias add + DMA out"""
                psp = ps2.tile(
                    [128, 512], F32, tag="psV", bufs=2, name=f"prj{ob}_{qc}"
                )
                for cb in range(CB):
                    nc.tensor.matmul(
                        psp[:, :],
                        wproj_sb[:, cb * C + ob * 128 : cb * C + (ob + 1) * 128],
                        attnT_sb[:, cb * NQ + qc * 512 : cb * NQ + (qc + 1) * 512],
                        start=(cb == 0),
                        stop=(cb == CB - 1),
                    )
                ot = op_pool.tile([128, 512], BF16, tag="out")
                nc.vector.tensor_scalar_add(
                    ot[:, :], psp[:, :], bias_sb[:, ob : ob + 1]
                )
                nc.sync.dma_start(
                    out=outT[ob * 128 : (ob + 1) * 128, qc * 512 : (qc + 1) * 512],
                    in_=ot[:, :],
                )

            # qc=1 projection: partial sums over pairs 0-4 ride as filler
            # in the last pass; only the pair-5 term + add/bias is tail
            projp_sb = [
                p23.tile([128, 512], F32, name=f"pp{ob}") for ob in range(CB)
            ]

            def proj_partial(ob):
                psp = ps2.tile(
                    [128, 512], F32, tag="psV", bufs=2, name=f"prjp{ob}"
                )
                for cb in range(CB - 1):
                    nc.tensor.matmul(
                        psp[:, :],
                        wproj_sb[:, cb * C + ob * 128 : cb * C + (ob + 1) * 128],
                        attnT_sb[:, cb * NQ + 512 : cb * NQ + 1024],
                        start=(cb == 0),
                        stop=(cb == CB - 2),
                    )
                nc.vector.tensor_copy(projp_sb[ob][:, :], psp[:, :])

            def proj_final(ob):
                cb = CB - 1
                psp = ps2.tile(
                    [128, 512], F32, tag="psV", bufs=2, name=f"prjf{ob}"
                )
                nc.tensor.matmul(
                    psp[:, :],
                    wproj_sb[:, cb * C + ob * 128 : cb * C + (ob + 1) * 128],
                    attnT_sb[:, cb * NQ + 512 : cb * NQ + 1024],
                    start=True,
                    stop=True,
                )
                ot = op_pool.tile([128, 512], BF16, tag="out")
                # (psp + bias) + partial, fused on DVE
                nc.vector.scalar_tensor_tensor(
                    ot[:, :],
                    psp[:, :],
                    bias_sb[:, ob : ob + 1],
                    projp_sb[ob][:, :],
                    op0=mybir.AluOpType.add,
                    op1=mybir.AluOpType.add,
                )
                nc.sync.dma_start(
                    out=outT[ob * 128 : (ob + 1) * 128, 512:1024],
                    in_=ot[:, :],
                )

            # ---- filler schedule --------------------------------------
            # filler[t] = list of closures to emit in step t's filler slot
            filler = [[] for _ in range(NSTEPS)]

            def _sched(t, fn):
                filler[t].append(fn)

            # remaining pair-0 k chunks first (their x chunks land before
            # the v weights): k tch needed by scores kb=4*tch (step 2*tch)
            for tch in range(1, 4):
                _sched(2 * (tch - 1), (lambda t=tch: kq_unit(0, t, False)))
            # v blocks just-in-time for pass 0: block kb in step kb//2
            for kb in range(TB):
                _sched(kb // 2, (lambda kb=kb: v_unit(kb, 0, 512)))
                _sched(kb // 2, (lambda kb=kb: v_unit(kb, 512, 256)))
            # pairs 1-5: k tch j needed by step 8*hb + 2*j, q tch0 by 8*hb
            for hb in range(1, HB):
                s0 = 8 * (hb - 1) + 3
                for j in range(4):
                    _sched(min(s0 + j, 8 * hb + 2 * j - 2),
                           (lambda h=hb, j=j: kq_unit(h, j, False)))
                _sched(8 * hb - 2, (lambda h=hb: kq_unit(h, 0, True)))
            # q tch1 units needed by pass (hb, 1) = step 48 + 8*hb
            for hb in range(HB):
                _sched(40 + hb, (lambda h=hb: kq_unit(h, 1, True)))
            # out-proj qc=0: attnT qc0 final after pass idx 5 epilogue
            # (multiply lands ~step 50); spread over the ACT-bound tail
            for ob in range(CB):
                _sched(54 + 4 * ob, (lambda o=ob: proj_unit(o, 0)))
            # out-proj qc=1 partials over pairs 0-4: pair-4 attnT qc1 is
            # multiplied at step 89 (pass idx 10 epilogue)
            for ob in range(CB):
                _sched(90 + ob, (lambda o=ob: proj_partial(o)))

            # ---- attention pipeline -----------------------------------
            def epi_pe(hb_, qc_, outs_):
                """PE part of a pass's normalization epilogue. The two
                heads' 1/denom broadcasts go to different column strips of
                one PSUM tile (col tiling) so they run concurrently."""
                psb = ps2.tile(
                    [128, 512], F32, tag="psV", bufs=2,
                    name=f"psb{hb_}_{qc_}",
                )
                for hh_ in range(2):
                    nc.tensor.matmul(
                        psb[64 * hh_ : 64 * hh_ + 64, :],
                        ones_sb[:, :],
                        outs_[hh_][1][:, :],
                        start=True,
                        stop=True,
                    )
                for hh_ in range(2):
                    nc.vector.tensor_mul(
                        attnT_sb[
                            64 * hh_ : 64 * hh_ + 64,
                            hb_ * NQ + qc_ * 512 : hb_ * NQ + (qc_ + 1) * 512,
                        ],
                        psb[64 * hh_ : 64 * hh_ + 64, :],
                        outs_[hh_][0][:, :],
                    )

            def emit_scores(hb, qc, kb2):
                """scores for k-blocks kb2, kb2+1 (both heads) -> psS
                tiles + exp -> probs; returns [(kb, probs), ...]"""
                q0 = hb * NQ + qc * 512
                out = []
                for kb in (kb2, kb2 + 1):
                    sc = ps2.tile(
                        [128, NQ], F32, tag="psS", bufs=2,
                        name=f"sc{hb}_{qc}_{kb}",
                    )
                    for hh in range(2):
                        p0 = 64 * hh
                        nc.tensor.matmul(
                            sc[:, hh * 512 : (hh + 1) * 512],
                            kT_sb[
                                p0 : p0 + 64,
                                hb * N + kb * 128 : hb * N + (kb + 1) * 128,
                            ],
                            qT_sb[p0 : p0 + 64, q0 : q0 + 512],
                            start=True,
                            stop=True,
                            tile_position=(p0, 0),
                        )
                    out.append((kb, sc))
                return out

            def emit_exps(pending_sc):
                out = []
                for kb, sc in pending_sc:
                    pb = hp.tile([128, NQ], BF16, tag="probs")
                    nc.scalar.activation(pb[:, :], sc[:, :], EXP, scale=SCALE)
                    out.append((kb, pb))
                return out

            def av_mms(accs, hb, pkb, ppb, heads=(0, 1)):
                for hh in heads:
                    vs = pkb * VW + (2 * hb + hh) * (D + 1)
                    nc.tensor.matmul(
                        accs[hh][0:65, :],
                        vaug_sb[:, vs : vs + D + 1],
                        ppb[:, hh * 512 : (hh + 1) * 512],
                        start=(pkb == 0),
                        stop=(pkb == TB - 1),
                    )

            def drain(accs):
                """epilogue DVE part: drain accumulators + 1/denominator"""
                outs = []
                for hh in range(2):
                    acc = accs[hh]
                    cpy = mi.tile([64, 512], F32, tag="cpy")
                    nc.vector.tensor_copy(cpy[:, :], acc[0:64, :])
                    den = mi.tile([1, 512], F32, tag="den")
                    nc.vector.tensor_copy(den[:, :], acc[64:65, :])
                    rec = mi.tile([1, 512], F32, tag="rec")
                    nc.vector.reciprocal_approx_fast(rec[:, :], den[:, :])
                    row = mi.tile([1, 512], BF16, tag="row")
                    nc.vector.tensor_copy(row[:, :], rec[:, :])
                    outs.append((cpy, row))
                return outs

            # ---- pre-phase: minimal (pair-0 k/q for the first chunk) --
            kq_unit(0, 0, False)
            kq_unit(0, 0, True)

            # ---- the flat pipeline ------------------------------------
            accs = None  # current pass's psA accumulators
            prev_probs = []  # [(kb, probs)] awaiting attn@v
            prev_pass = None  # (hb, qc) owning prev_probs
            pend_epi = None  # (hb, qc, outs, due_step)

            def new_accs(hb, qc):
                return [
                    ps2.tile(
                        [128, 512], F32, tag="psA", bufs=2,
                        name=f"acc{hb}_{qc}_{i}",
                    )
                    for i in range(2)
                ]

            for t in range(NSTEPS):
                hb, qc = PASSES[t // 8]
                kb2 = (t % 8) * 2
                # scores first: keeps ACT fed across pass boundaries
                pend_sc = emit_scores(hb, qc, kb2)
                # attn@v for the previous step's probs
                for pkb, ppb in prev_probs:
                    av_mms(accs, prev_pass[0], pkb, ppb)
                if prev_probs and prev_probs[-1][0] == TB - 1:
                    # previous pass complete: drain + defer its epilogue,
                    # then recycle the acc slots for the new pass
                    outs = drain(accs)
                    pend_epi = (prev_pass[0], prev_pass[1], outs, t + 1)
                    accs = new_accs(hb, qc)
                elif accs is None:
                    accs = new_accs(hb, qc)
                # filler work
                for fn in filler[t]:
                    fn()
                # deferred epilogue of the pass before last
                if pend_epi is not None and t >= pend_epi[3]:
                    epi_pe(pend_epi[0], pend_epi[1], pend_epi[2])
                    pend_epi = None
                # exp of this step's scores
                prev_probs = emit_exps(pend_sc)
                prev_pass = (hb, qc)

            # ---- tail: last pass's attn@v drain + epilogue ------------
            if pend_epi is not None:
                epi_pe(pend_epi[0], pend_epi[1], pend_epi[2])
                pend_epi = None
            for pkb, ppb in prev_probs:
                av_mms(accs, prev_pass[0], pkb, ppb)
            outs = drain(accs)
            epi_pe(prev_pass[0], prev_pass[1], outs)

            # ---- output projection, qc=1 final terms ------------------
            for ob in range(CB):
                proj_final(ob)

    nc.finalize()
    return nc


_NC_CACHE = []


def _get_nc():
    if not _NC_CACHE:
        _NC_CACHE.append(_build())
    return _NC_CACHE[0]


def kernel(x, w_qkv, w_proj, b_proj):
    x = np.asarray(x, dtype=np.float32)
    w_qkv = np.asarray(w_qkv, dtype=np.float32)
    w_proj = np.asarray(w_proj, dtype=np.float32)
    b_proj = np.asarray(b_proj, dtype=np.float32)

    nc = _get_nc()

    wqkvT = w_qkv.T.astype(ml_dtypes.bfloat16)  # [C, 3C]
    wq3 = np.ascontiguousarray(wqkvT).reshape(CB, 128, 3 * C)  # [ci, p, o]
    wqp = np.concatenate(
        [
            wq3[:, :, o0 : o0 + w].transpose(1, 0, 2).reshape(128, CB * w)
            for o0, w in _WQ_GROUPS
        ],
        axis=1,
    )
    wqp = np.ascontiguousarray(wqp)
    # SBUF images: wproj cols = [ci][o], bias cols = [ci]
    wprojp = np.ascontiguousarray(
        w_proj.T.astype(ml_dtypes.bfloat16).reshape(CB, 128, C)
        .transpose(1, 0, 2)
        .reshape(128, CB * C)
    )
    biasp = np.ascontiguousarray(
        b_proj.astype(np.float32).reshape(CB, 128).T
    )

    in_maps = []
    for core in range(N_CORES):
        b, half = divmod(core, 2)
        # own 1024 query tokens first, then the other half (key order
        # within attention is permutation-invariant)
        mine = x[b, half * NQ : (half + 1) * NQ].T
        other = x[b, (1 - half) * NQ : (2 - half) * NQ].T
        xTc = np.concatenate([mine, other], axis=1).astype(ml_dtypes.bfloat16)
        # pack to the SBUF image: cols = [tch][ci][t]
        xTp = np.ascontiguousarray(
            xTc.reshape(CB, 128, 4, 512).transpose(1, 2, 0, 3).reshape(128, CB * N)
        )
        in_maps.append({"xTp": xTp, "wqp": wqp, "wprojp": wprojp, "biasp": biasp})

    res = run_bass_kernel_spmd(nc, in_maps, core_ids=list(range(N_CORES)))

    out = np.empty((B, N, C), dtype=np.float32)
    for core in range(N_CORES):
        b, half = divmod(core, 2)
        out[b, half * NQ : (half + 1) * NQ, :] = (
            res.results[core]["outT"].astype(np.float32).T
        )
    return out


# revision 14
# speedup vs baseline: 1.1937x; 1.0017x over previous
"""Distributed multi-head attention for TRN2 (8 NeuronCores).

Reference computation (per batch b):
    qkv = x @ w_qkv.T                         # (N, 3C)
    q, k, v = split/reshape to (H, N, D)
    attn = softmax(q @ k.T * D**-0.5)         # per head
    out = (attn @ v) reassembled to (N, C)
    out = out @ w_proj.T + b_proj

Sharding: 8 cores = 4 batches x 2 query-halves. Each core computes k/v
for all 2048 tokens of its batch (duplicated across the 2 cores of a
batch - cheaper than communicating), q for its own 1024 tokens, the
full attention for all 12 heads over its 1024 queries, and the output
projection. No collectives.

Layout strategy (all chosen so no on-chip transposes are needed):
  - host passes x^T and w_qkv^T so projections contract over partitions
  - q,k are produced "d-major" ([head-dim, tokens]) via out^T-form
    matmuls; scores are computed transposed ([keys, queries]) which is
    exactly the layout attn@v consumes as its stationary-side operand
  - softmax needs no max-subtraction (scores ~ N(0,1), fp32 exp range)
  - the denominator rides along as a ones-column appended to v (M=65
    matmuls); normalization uses a K=1 ones-matmul to broadcast 1/denom
    across partitions
  - all matmuls in bf16 (PSUM accumulation is fp32); softmax exp runs
    on the scalar (ACT) engine from PSUM f32, writing bf16 probs

Schedule: one flat software pipeline over 96 score-steps (12 passes x
8 k-block pairs), passes ordered qc-major. Scores for step t+1 are
emitted before step t's attn@v so the ACT engine (the per-step floor,
~2.1us/step) never waits at pass boundaries. Everything else -- the
qkv projections, and the output projection for the first query half --
rides in deadline-scheduled filler slots; out-proj qc=0 fills the
otherwise ACT-bound back half of the pipeline. The pre-phase is just 2
projection units, and the startup DMAs are chunked per 128x512 tile so
the first matmul fires as soon as its first chunk lands.

Self-contained: hardcodes B=4, N=2048, C=768, H=12, D=64.
"""

import numpy as np
import ml_dtypes

import concourse.bass as bass
import concourse.mybir as mybir
from concourse import bacc
from concourse.tile import TileContext
from concourse.bass_utils import run_bass_kernel_spmd

F32 = mybir.dt.float32
BF16 = mybir.dt.bfloat16
EXP = mybir.ActivationFunctionType.Exp

B, N, C = 4, 2048, 768
H, D = 12, 64
SCALE = float(D) ** -0.5  # 0.125
NQ = N // 2  # queries per core: 1024
CB = C // 128  # 6 c-chunks
TB = N // 128  # 16 token blocks
HB = H // 2  # 6 head pairs
VW = H * (D + 1)  # 780: v block width with ones columns

N_CORES = 8

# w_qkv columns, grouped in the order the projection units consume them:
# pair-0 k/q, v (split 512+256 for finer DMA deps), then k/q for pairs
# 1..5. Each group holds its column range for all six 128-row input
# chunks, contiguously.
_WQ_GROUPS = [(C, 128), (0, 128), (2 * C, 512), (2 * C + 512, 256)]
for _ob in range(1, CB):
    _WQ_GROUPS.append((C + _ob * 128, 128))
    _WQ_GROUPS.append((_ob * 128, 128))
_WQ_BASE = {}
_cur = 0
for _o0, _w in _WQ_GROUPS:
    _WQ_BASE[_o0] = (_cur, _w)
    _cur += CB * _w

# pass order: qc-major so attnT for qc=0 is complete by mid-pipeline and
# the qc=0 output projection can fill the ACT-bound later passes
PASSES = [(hb, 0) for hb in range(HB)] + [(hb, 1) for hb in range(HB)]
NSTEPS = len(PASSES) * 8  # 96 score-steps, 2 k-blocks each


def _build():
    nc = bacc.Bacc(None, target_bir_lowering=False)

    # host-packed SBUF images: xTp cols = [tch][ci][t]; wqp cols grouped
    # in consumption order (see _WQ_GROUPS)
    xTp = nc.declare_dram_parameter("xTp", [128, CB * N], BF16, isOutput=False)
    wqp = nc.declare_dram_parameter("wqp", [128, CB * 3 * C], BF16, isOutput=False)
    wprojp = nc.declare_dram_parameter("wprojp", [128, CB * C], BF16, isOutput=False)
    biasp = nc.declare_dram_parameter("biasp", [128, CB], F32, isOutput=False)
    outT = nc.declare_dram_parameter("outT", [C, NQ], BF16, isOutput=True)

    with TileContext(nc) as tc:
        with (
            tc.tile_pool(name="per", bufs=1) as per,
            tc.tile_pool(name="p23", bufs=1) as p23,
            tc.tile_pool(name="hp", bufs=8) as hp,
            tc.tile_pool(name="mi", bufs=3) as mi,
            tc.tile_pool(name="op", bufs=2) as op_pool,
            tc.tile_pool(name="wq", bufs=1) as wq_pool,
            tc.tile_pool(name="xt", bufs=1) as xt_pool,
            tc.tile_pool(name="ps", bufs=2, space="PSUM") as ps2,
        ):
            # ---- persistent tiles -------------------------------------
            qT_sb = per.tile([128, CB * NQ], BF16)  # q^T  [2 heads/blk, 1024]
            kT_sb = per.tile([128, CB * N], BF16)  # k^T  [2 heads/blk, 2048]
            vaug_sb = per.tile([128, TB * VW], BF16)  # v + ones cols
            bias_sb = per.tile([128, CB], F32)
            ones_sb = per.tile([1, 64], BF16)
            attnT_sb = p23.tile([128, CB * NQ], BF16)  # attn out^T
            wproj_sb = p23.tile([128, CB * C], BF16)

            nc.vector.memset(ones_sb[:, :], 1.0)
            # ones columns of vaug: col 64 of each 65-wide head slot
            vaug_ones = vaug_sb[:, :].rearrange(
                "p (t h x) -> p t h x", t=TB, h=H, x=D + 1
            )[:, :, :, D : D + 1]
            nc.vector.memset(vaug_ones, 1.0)

            wqkv_sb = wq_pool.tile([128, CB * 3 * C], BF16)
            # x chunks as separate [128, 512] tiles per (tch, ci) so
            # startup DMA deps are fine-grained
            xts = [
                [
                    xt_pool.tile([128, 512], BF16, name=f"xt{t}_{ci}")
                    for ci in range(CB)
                ]
                for t in range(4)
            ]

            def _dma_xt_ci(tch, ci):
                base = tch * CB * 512 + ci * 512
                nc.sync.dma_start(
                    out=xts[tch][ci][:, :], in_=xTp[:, base : base + 512]
                )

            def _dma_wq(gi, ci=None):
                o0, w = _WQ_GROUPS[gi]
                base, _ = _WQ_BASE[o0]
                if ci is None:
                    nc.sync.dma_start(
                        out=wqkv_sb[:, base : base + CB * w],
                        in_=wqp[:, base : base + CB * w],
                    )
                else:
                    b0 = base + ci * w
                    nc.sync.dma_start(
                        out=wqkv_sb[:, b0 : b0 + w], in_=wqp[:, b0 : b0 + w]
                    )

            # DMA priority order (first-consumption order):
            #   pair0-k & x-chunk0 (pre-phase), pair0-q, x-chunk1 (k filler),
            #   v weights, x-chunks 2-3, later pairs' k/q, bias/wproj
            for ci in range(CB):
                _dma_wq(0, ci)
                _dma_xt_ci(0, ci)
            for ci in range(CB):
                _dma_wq(1, ci)
            for ci in range(CB):
                _dma_xt_ci(1, ci)
            for ci in range(CB):
                _dma_wq(2, ci)
            for ci in range(CB):
                _dma_wq(3, ci)
            for ci in range(CB):
                _dma_xt_ci(2, ci)
            for ci in range(CB):
                _dma_xt_ci(3, ci)
            for gi in range(4, len(_WQ_GROUPS)):
                _dma_wq(gi)
            nc.sync.dma_start(out=bias_sb[:, :], in_=biasp[:, :])
            nc.sync.dma_start(out=wproj_sb[:, :], in_=wprojp[:, :])

            def wq(ci, o0, width):
                base, gw = _WQ_BASE[o0]
                return wqkv_sb[:, base + ci * gw : base + ci * gw + width]

            # ---- projection work units (PE filler) --------------------
            def kq_unit(ob, tch, is_q):
                """one k^T (or q^T) block: out-dims block ob, 512 tokens"""
                t0 = tch * 512
                kind = "q" if is_q else "k"
                psv = ps2.tile(
                    [128, 512], F32, tag="psV", bufs=2, name=f"{kind}{ob}_{tch}"
                )
                for ci in range(CB):
                    nc.tensor.matmul(
                        psv[:, :],
                        wq(ci, (0 if is_q else C) + ob * 128, 128),
                        xts[tch][ci][:, :],
                        start=(ci == 0),
                        stop=(ci == CB - 1),
                    )
                if is_q:
                    nc.vector.tensor_copy(
                        qT_sb[:, ob * NQ + t0 : ob * NQ + t0 + 512], psv[:, :]
                    )
                else:
                    nc.vector.tensor_copy(
                        kT_sb[:, ob * N + t0 : ob * N + t0 + 512], psv[:, :]
                    )

            def v_unit(t128, o0, w):
                """one v unit: 128 tokens x [o0, o0+w) v-dims, written
                (bf16) into the vaug slot layout"""
                tch, tb = divmod(t128, 4)
                psv = ps2.tile(
                    [128, 512], F32, tag="psV", bufs=2, name=f"v{t128}_{o0}"
                )
                for ci in range(CB):
                    nc.tensor.matmul(
                        psv[:, :w],
                        xts[tch][ci][:, tb * 128 : (tb + 1) * 128],
                        wq(ci, 2 * C + o0, w),
                        start=(ci == 0),
                        stop=(ci == CB - 1),
                    )
                nh = w // D
                src = psv[:, :w].rearrange("p (h x) -> p h x", x=D)
                h0 = o0 // D
                base = t128 * VW + h0 * (D + 1)
                dst = vaug_sb[:, base : base + nh * (D + 1)].rearrange(
                    "p (h x) -> p h x", x=D + 1
                )[:, :, :D]
                nc.vector.tensor_copy(dst, src)

            def proj_unit(ob, qc):
                """output projection for out-dims block ob, query half qc
                (contracts all 6 attnT pair-blocks), bias add + DMA out"""
                psp = ps2.tile(
                    [128, 512], F32, tag="psV", bufs=2, name=f"prj{ob}_{qc}"
                )
                for cb in range(CB):
                    nc.tensor.matmul(
                        psp[:, :],
                        wproj_sb[:, cb * C + ob * 128 : cb * C + (ob + 1) * 128],
                        attnT_sb[:, cb * NQ + qc * 512 : cb * NQ + (qc + 1) * 512],
                        start=(cb == 0),
                        stop=(cb == CB - 1),
                    )
                ot = op_pool.tile([128, 512], BF16, tag="out")
                nc.vector.tensor_scalar_add(
                    ot[:, :], psp[:, :], bias_sb[:, ob : ob + 1]
                )
                nc.sync.dma_start(
                    out=outT[ob * 128 : (ob + 1) * 128, qc * 512 : (qc + 1) * 512],
                    in_=ot[:, :],
                )

            # qc=1 projection: partial sums over pairs 0-4 ride as filler
            # in the last pass; only the pair-5 term + add/bias is tail
            projp_sb = [
                p23.tile([128, 512], F32, name=f"pp{ob}") for ob in range(CB)
            ]

            def proj_partial(ob):
                psp = ps2.tile(
                    [128, 512], F32, tag="psV", bufs=2, name=f"prjp{ob}"
                )
                for cb in range(CB - 1):
                    nc.tensor.matmul(
                        psp[:, :],
                        wproj_sb[:, cb * C + ob * 128 : cb * C + (ob + 1) * 128],
                        attnT_sb[:, cb * NQ + 512 : cb * NQ + 1024],
                        start=(cb == 0),
                        stop=(cb == CB - 2),
                    )
                nc.vector.tensor_copy(projp_sb[ob][:, :], psp[:, :])

            def proj_final(ob):
                cb = CB - 1
                psp = ps2.tile(
                    [128, 512], F32, tag="psV", bufs=2, name=f"prjf{ob}"
                )
                nc.tensor.matmul(
                    psp[:, :],
                    wproj_sb[:, cb * C + ob * 128 : cb * C + (ob + 1) * 128],
                    attnT_sb[:, cb * NQ + 512 : cb * NQ + 1024],
                    start=True,
                    stop=True,
                )
                ot = op_pool.tile([128, 512], BF16, tag="out")
                # (psp + bias) + partial, fused on DVE
                nc.vector.scalar_tensor_tensor(
                    ot[:, :],
                    psp[:, :],
                    bias_sb[:, ob : ob + 1],
                    projp_sb[ob][:, :],
                    op0=mybir.AluOpType.add,
                    op1=mybir.AluOpType.add,
                )
                nc.sync.dma_start(
                    out=outT[ob * 128 : (ob + 1) * 128, 512:1024],
                    in_=ot[:, :],
                )

            # ---- filler schedule --------------------------------------
            # filler[t] = list of closures to emit in step t's filler slot
            filler = [[] for _ in range(NSTEPS)]

            def _sched(t, fn):
                filler[t].append(fn)

            # remaining pair-0 k chunks first (their x chunks land before
            # the v weights): k tch needed by scores kb=4*tch (step 2*tch)
            for tch in range(1, 4):
                _sched(2 * (tch - 1), (lambda t=tch: kq_unit(0, t, False)))
            # v blocks just-in-time for pass 0: block kb in step kb//2
            for kb in range(TB):
                _sched(kb // 2, (lambda kb=kb: v_unit(kb, 0, 512)))
                _sched(kb // 2, (lambda kb=kb: v_unit(kb, 512, 256)))
            # pairs 1-5: k tch j needed by step 8*hb + 2*j, q tch0 by 8*hb
            for hb in range(1, HB):
                s0 = 8 * (hb - 1) + 3
                for j in range(4):
                    _sched(min(s0 + j, 8 * hb + 2 * j - 2),
                           (lambda h=hb, j=j: kq_unit(h, j, False)))
                _sched(8 * hb - 2, (lambda h=hb: kq_unit(h, 0, True)))
            # q tch1 units needed by pass (hb, 1) = step 48 + 8*hb
            for hb in range(HB):
                _sched(40 + hb, (lambda h=hb: kq_unit(h, 1, True)))
            # out-proj qc=0: attnT qc0 final after pass idx 5 epilogue
            # (multiply lands ~step 50); spread over the ACT-bound tail
            for ob in range(CB):
                _sched(54 + 4 * ob, (lambda o=ob: proj_unit(o, 0)))
            # out-proj qc=1 partials over pairs 0-4: pair-4 attnT qc1 is
            # multiplied at step 89 (pass idx 10 epilogue)
            for ob in range(CB):
                _sched(90 + ob, (lambda o=ob: proj_partial(o)))

            # ---- attention pipeline -----------------------------------
            def epi_pe(hb_, qc_, outs_):
                """PE part of a pass's normalization epilogue. The two
                heads' 1/denom broadcasts go to different column strips of
                one PSUM tile (col tiling) so they run concurrently."""
                psb = ps2.tile(
                    [128, 512], F32, tag="psV", bufs=2,
                    name=f"psb{hb_}_{qc_}",
                )
                for hh_ in range(2):
                    nc.tensor.matmul(
                        psb[64 * hh_ : 64 * hh_ + 64, :],
                        ones_sb[:, :],
                        outs_[hh_][1][:, :],
                        start=True,
                        stop=True,
                    )
                for hh_ in range(2):
                    nc.vector.tensor_mul(
                        attnT_sb[
                            64 * hh_ : 64 * hh_ + 64,
                            hb_ * NQ + qc_ * 512 : hb_ * NQ + (qc_ + 1) * 512,
                        ],
                        psb[64 * hh_ : 64 * hh_ + 64, :],
                        outs_[hh_][0][:, :],
                    )

            def emit_scores(hb, qc, kb2):
                """scores for k-blocks kb2, kb2+1 (both heads) -> psS
                tiles + exp -> probs; returns [(kb, probs), ...]"""
                q0 = hb * NQ + qc * 512
                out = []
                for kb in (kb2, kb2 + 1):
                    sc = ps2.tile(
                        [128, NQ], F32, tag="psS", bufs=2,
                        name=f"sc{hb}_{qc}_{kb}",
                    )
                    for hh in range(2):
                        p0 = 64 * hh
                        nc.tensor.matmul(
                            sc[:, hh * 512 : (hh + 1) * 512],
                            kT_sb[
                                p0 : p0 + 64,
                                hb * N + kb * 128 : hb * N + (kb + 1) * 128,
                            ],
                            qT_sb[p0 : p0 + 64, q0 : q0 + 512],
                            start=True,
                            stop=True,
                            tile_position=(p0, 0),
                        )
                    out.append((kb, sc))
                return out

            def emit_exps(pending_sc):
                out = []
                for kb, sc in pending_sc:
                    pb = hp.tile([128, NQ], BF16, tag="probs")
                    nc.scalar.activation(pb[:, :], sc[:, :], EXP, scale=SCALE)
                    out.append((kb, pb))
                return out

            def av_mms(accs, hb, pkb, ppb, heads=(0, 1)):
                for hh in heads:
                    vs = pkb * VW + (2 * hb + hh) * (D + 1)
                    nc.tensor.matmul(
                        accs[hh][0:65, :],
                        vaug_sb[:, vs : vs + D + 1],
                        ppb[:, hh * 512 : (hh + 1) * 512],
                        start=(pkb == 0),
                        stop=(pkb == TB - 1),
                    )

            def drain(accs):
                """epilogue: drain accumulators (on the otherwise-idle
                Pool engine) + 1/denominator (DVE)"""
                outs = []
                for hh in range(2):
                    acc = accs[hh]
                    cpy = mi.tile([64, 512], F32, tag="cpy")
                    nc.vector.tensor_copy(cpy[:, :], acc[0:64, :])
                    den = mi.tile([1, 512], F32, tag="den")
                    nc.vector.tensor_copy(den[:, :], acc[64:65, :])
                    rec = mi.tile([1, 512], F32, tag="rec")
                    nc.vector.reciprocal_approx_fast(rec[:, :], den[:, :])
                    row = mi.tile([1, 512], BF16, tag="row")
                    nc.vector.tensor_copy(row[:, :], rec[:, :])
                    outs.append((cpy, row))
                return outs

            # ---- pre-phase: minimal (pair-0 k/q for the first chunk) --
            kq_unit(0, 0, False)
            kq_unit(0, 0, True)

            # ---- the flat pipeline ------------------------------------
            accs = None  # current pass's psA accumulators
            prev_probs = []  # [(kb, probs)] awaiting attn@v
            prev_pass = None  # (hb, qc) owning prev_probs
            pend_epi = None  # (hb, qc, outs, due_step)

            def new_accs(hb, qc):
                return [
                    ps2.tile(
                        [128, 512], F32, tag="psA", bufs=2,
                        name=f"acc{hb}_{qc}_{i}",
                    )
                    for i in range(2)
                ]

            for t in range(NSTEPS):
                hb, qc = PASSES[t // 8]
                kb2 = (t % 8) * 2
                # scores first: keeps ACT fed across pass boundaries
                pend_sc = emit_scores(hb, qc, kb2)
                # attn@v for the previous step's probs
                for pkb, ppb in prev_probs:
                    av_mms(accs, prev_pass[0], pkb, ppb)
                if prev_probs and prev_probs[-1][0] == TB - 1:
                    # previous pass complete: drain + defer its epilogue,
                    # then recycle the acc slots for the new pass
                    outs = drain(accs)
                    pend_epi = (prev_pass[0], prev_pass[1], outs, t + 1)
                    accs = new_accs(hb, qc)
                elif accs is None:
                    accs = new_accs(hb, qc)
                # filler work
                for fn in filler[t]:
                    fn()
                # deferred epilogue of the pass before last
                if pend_epi is not None and t >= pend_epi[3]:
                    epi_pe(pend_epi[0], pend_epi[1], pend_epi[2])
                    pend_epi = None
                # exp of this step's scores
                prev_probs = emit_exps(pend_sc)
                prev_pass = (hb, qc)

            # ---- tail: last pass's attn@v drain + epilogue ------------
            if pend_epi is not None:
                epi_pe(pend_epi[0], pend_epi[1], pend_epi[2])
                pend_epi = None
            for pkb, ppb in prev_probs:
                av_mms(accs, prev_pass[0], pkb, ppb)
            # drain on ACT (idle after the final exp) + DVE for recip
            outs = []
            for hh in range(2):
                acc = accs[hh]
                cpy = mi.tile([64, 512], F32, tag="cpy")
                nc.scalar.copy(cpy[:, :], acc[0:64, :])
                den = mi.tile([1, 512], F32, tag="den")
                nc.scalar.copy(den[:, :], acc[64:65, :])
                rec = mi.tile([1, 512], F32, tag="rec")
                nc.vector.reciprocal_approx_fast(rec[:, :], den[:, :])
                row = mi.tile([1, 512], BF16, tag="row")
                nc.vector.tensor_copy(row[:, :], rec[:, :])
                outs.append((cpy, row))
            epi_pe(prev_pass[0], prev_pass[1], outs)

            # ---- output projection, qc=1 final terms ------------------
            for ob in range(CB):
                proj_final(ob)

    nc.finalize()
    return nc


_NC_CACHE = []


def _get_nc():
    if not _NC_CACHE:
        _NC_CACHE.append(_build())
    return _NC_CACHE[0]


def kernel(x, w_qkv, w_proj, b_proj):
    x = np.asarray(x, dtype=np.float32)
    w_qkv = np.asarray(w_qkv, dtype=np.float32)
    w_proj = np.asarray(w_proj, dtype=np.float32)
    b_proj = np.asarray(b_proj, dtype=np.float32)

    nc = _get_nc()

    wqkvT = w_qkv.T.astype(ml_dtypes.bfloat16)  # [C, 3C]
    wq3 = np.ascontiguousarray(wqkvT).reshape(CB, 128, 3 * C)  # [ci, p, o]
    wqp = np.concatenate(
        [
            wq3[:, :, o0 : o0 + w].transpose(1, 0, 2).reshape(128, CB * w)
            for o0, w in _WQ_GROUPS
        ],
        axis=1,
    )
    wqp = np.ascontiguousarray(wqp)
    # SBUF images: wproj cols = [ci][o], bias cols = [ci]
    wprojp = np.ascontiguousarray(
        w_proj.T.astype(ml_dtypes.bfloat16).reshape(CB, 128, C)
        .transpose(1, 0, 2)
        .reshape(128, CB * C)
    )
    biasp = np.ascontiguousarray(
        b_proj.astype(np.float32).reshape(CB, 128).T
    )

    in_maps = []
    for core in range(N_CORES):
        b, half = divmod(core, 2)
        # own 1024 query tokens first, then the other half (key order
        # within attention is permutation-invariant)
        mine = x[b, half * NQ : (half + 1) * NQ].T
        other = x[b, (1 - half) * NQ : (2 - half) * NQ].T
        xTc = np.concatenate([mine, other], axis=1).astype(ml_dtypes.bfloat16)
        # pack to the SBUF image: cols = [tch][ci][t]
        xTp = np.ascontiguousarray(
            xTc.reshape(CB, 128, 4, 512).transpose(1, 2, 0, 3).reshape(128, CB * N)
        )
        in_maps.append({"xTp": xTp, "wqp": wqp, "wprojp": wprojp, "biasp": biasp})

    res = run_bass_kernel_spmd(nc, in_maps, core_ids=list(range(N_CORES)))

    out = np.empty((B, N, C), dtype=np.float32)
    for core in range(N_CORES):
        b, half = divmod(core, 2)
        out[b, half * NQ : (half + 1) * NQ, :] = (
            res.results[core]["outT"].astype(np.float32).T
        )
    return out


# revision 15
# speedup vs baseline: 1.2027x; 1.0075x over previous
"""Distributed multi-head attention for TRN2 (8 NeuronCores).

Reference computation (per batch b):
    qkv = x @ w_qkv.T                         # (N, 3C)
    q, k, v = split/reshape to (H, N, D)
    attn = softmax(q @ k.T * D**-0.5)         # per head
    out = (attn @ v) reassembled to (N, C)
    out = out @ w_proj.T + b_proj

Sharding: 8 cores = 4 batches x 2 query-halves. Each core computes k/v
for all 2048 tokens of its batch (duplicated across the 2 cores of a
batch - cheaper than communicating), q for its own 1024 tokens, the
full attention for all 12 heads over its 1024 queries, and the output
projection. No collectives.

Layout strategy (all chosen so no on-chip transposes are needed):
  - host passes x^T and w_qkv^T so projections contract over partitions
  - q,k are produced "d-major" ([head-dim, tokens]) via out^T-form
    matmuls; scores are computed transposed ([keys, queries]) which is
    exactly the layout attn@v consumes as its stationary-side operand
  - softmax needs no max-subtraction (scores ~ N(0,1), fp32 exp range)
  - the denominator rides along as a ones-column appended to v (M=65
    matmuls); normalization uses a K=1 ones-matmul to broadcast 1/denom
    across partitions
  - all matmuls in bf16 (PSUM accumulation is fp32); softmax exp runs
    on the scalar (ACT) engine from PSUM f32, writing bf16 probs

Schedule: one flat software pipeline over 96 score-steps (12 passes x
8 k-block pairs), passes ordered qc-major. Scores for step t+1 are
emitted before step t's attn@v so the ACT engine (the per-step floor,
~2.1us/step) never waits at pass boundaries. Everything else -- the
qkv projections, and the output projection for the first query half --
rides in deadline-scheduled filler slots; out-proj qc=0 fills the
otherwise ACT-bound back half of the pipeline. The pre-phase is just 2
projection units, and the startup DMAs are chunked per 128x512 tile so
the first matmul fires as soon as its first chunk lands.

Self-contained: hardcodes B=4, N=2048, C=768, H=12, D=64.
"""

import numpy as np
import ml_dtypes

import concourse.bass as bass
import concourse.mybir as mybir
from concourse import bacc
from concourse.tile import TileContext
from concourse.bass_utils import run_bass_kernel_spmd

F32 = mybir.dt.float32
BF16 = mybir.dt.bfloat16
EXP = mybir.ActivationFunctionType.Exp

B, N, C = 4, 2048, 768
H, D = 12, 64
SCALE = float(D) ** -0.5  # 0.125
NQ = N // 2  # queries per core: 1024
CB = C // 128  # 6 c-chunks
TB = N // 128  # 16 token blocks
HB = H // 2  # 6 head pairs
VW = H * (D + 1)  # 780: v block width with ones columns

N_CORES = 8

# w_qkv columns, grouped in the order the projection units consume them:
# pair-0 k/q, v (split 512+256 for finer DMA deps), then k/q for pairs
# 1..5. Each group holds its column range for all six 128-row input
# chunks, contiguously.
_WQ_GROUPS = [(C, 128), (0, 128), (2 * C, 512), (2 * C + 512, 256)]
for _ob in range(1, CB):
    _WQ_GROUPS.append((C + _ob * 128, 128))
    _WQ_GROUPS.append((_ob * 128, 128))
_WQ_BASE = {}
_cur = 0
for _o0, _w in _WQ_GROUPS:
    _WQ_BASE[_o0] = (_cur, _w)
    _cur += CB * _w

# pass order: qc-major so attnT for qc=0 is complete by mid-pipeline and
# the qc=0 output projection can fill the ACT-bound later passes
PASSES = [(hb, 0) for hb in range(HB)] + [(hb, 1) for hb in range(HB)]
NSTEPS = len(PASSES) * 8  # 96 score-steps, 2 k-blocks each


def _build():
    nc = bacc.Bacc(None, target_bir_lowering=False)

    # host-packed SBUF images: xTp cols = [tch][ci][t]; wqp cols grouped
    # in consumption order (see _WQ_GROUPS)
    xTp = nc.declare_dram_parameter("xTp", [128, CB * N], BF16, isOutput=False)
    wqp = nc.declare_dram_parameter("wqp", [128, CB * 3 * C], BF16, isOutput=False)
    wprojp = nc.declare_dram_parameter("wprojp", [128, CB * C], BF16, isOutput=False)
    biasp = nc.declare_dram_parameter("biasp", [128, CB], F32, isOutput=False)
    outT = nc.declare_dram_parameter("outT", [C, NQ], BF16, isOutput=True)

    with TileContext(nc) as tc:
        with (
            tc.tile_pool(name="per", bufs=1) as per,
            tc.tile_pool(name="p23", bufs=1) as p23,
            tc.tile_pool(name="hp", bufs=8) as hp,
            tc.tile_pool(name="mi", bufs=3) as mi,
            tc.tile_pool(name="op", bufs=2) as op_pool,
            tc.tile_pool(name="wq", bufs=1) as wq_pool,
            tc.tile_pool(name="xt", bufs=1) as xt_pool,
            tc.tile_pool(name="ps", bufs=2, space="PSUM") as ps2,
        ):
            # ---- persistent tiles -------------------------------------
            qT_sb = per.tile([128, CB * NQ], BF16)  # q^T  [2 heads/blk, 1024]
            kT_sb = per.tile([128, CB * N], BF16)  # k^T  [2 heads/blk, 2048]
            vaug_sb = per.tile([128, TB * VW], BF16)  # v + ones cols
            bias_sb = per.tile([128, CB], F32)
            ones_sb = per.tile([1, 64], BF16)
            attnT_sb = p23.tile([128, CB * NQ], BF16)  # attn out^T
            wproj_sb = p23.tile([128, CB * C], BF16)

            nc.vector.memset(ones_sb[:, :], 1.0)
            # ones columns of vaug: col 64 of each 65-wide head slot
            vaug_ones = vaug_sb[:, :].rearrange(
                "p (t h x) -> p t h x", t=TB, h=H, x=D + 1
            )[:, :, :, D : D + 1]
            nc.vector.memset(vaug_ones, 1.0)

            wqkv_sb = wq_pool.tile([128, CB * 3 * C], BF16)
            # x chunks as separate [128, 512] tiles per (tch, ci) so
            # startup DMA deps are fine-grained
            xts = [
                [
                    xt_pool.tile([128, 512], BF16, name=f"xt{t}_{ci}")
                    for ci in range(CB)
                ]
                for t in range(4)
            ]

            def _dma_xt_ci(tch, ci):
                base = tch * CB * 512 + ci * 512
                nc.sync.dma_start(
                    out=xts[tch][ci][:, :], in_=xTp[:, base : base + 512]
                )

            def _dma_wq(gi, ci=None):
                o0, w = _WQ_GROUPS[gi]
                base, _ = _WQ_BASE[o0]
                if ci is None:
                    nc.sync.dma_start(
                        out=wqkv_sb[:, base : base + CB * w],
                        in_=wqp[:, base : base + CB * w],
                    )
                else:
                    b0 = base + ci * w
                    nc.sync.dma_start(
                        out=wqkv_sb[:, b0 : b0 + w], in_=wqp[:, b0 : b0 + w]
                    )

            # DMA priority order (first-consumption order):
            #   pair0-k & x-chunk0 (pre-phase), pair0-q, x-chunk1 (k filler),
            #   v weights, x-chunks 2-3, later pairs' k/q, bias/wproj
            for ci in range(CB):
                _dma_wq(0, ci)
                _dma_xt_ci(0, ci)
            for ci in range(CB):
                _dma_wq(1, ci)
            for ci in range(CB):
                _dma_xt_ci(1, ci)
            for ci in range(CB):
                _dma_wq(2, ci)
            for ci in range(CB):
                _dma_wq(3, ci)
            for ci in range(CB):
                _dma_xt_ci(2, ci)
            for ci in range(CB):
                _dma_xt_ci(3, ci)
            for gi in range(4, len(_WQ_GROUPS)):
                _dma_wq(gi)
            nc.sync.dma_start(out=bias_sb[:, :], in_=biasp[:, :])
            nc.sync.dma_start(out=wproj_sb[:, :], in_=wprojp[:, :])

            def wq(ci, o0, width):
                base, gw = _WQ_BASE[o0]
                return wqkv_sb[:, base + ci * gw : base + ci * gw + width]

            # ---- projection work units (PE filler) --------------------
            def kq_unit(ob, tch, is_q):
                """one k^T (or q^T) block: out-dims block ob, 512 tokens"""
                t0 = tch * 512
                kind = "q" if is_q else "k"
                psv = ps2.tile(
                    [128, 512], F32, tag="psV", bufs=2, name=f"{kind}{ob}_{tch}"
                )
                for ci in range(CB):
                    nc.tensor.matmul(
                        psv[:, :],
                        wq(ci, (0 if is_q else C) + ob * 128, 128),
                        xts[tch][ci][:, :],
                        start=(ci == 0),
                        stop=(ci == CB - 1),
                    )
                if is_q:
                    nc.vector.tensor_copy(
                        qT_sb[:, ob * NQ + t0 : ob * NQ + t0 + 512], psv[:, :]
                    )
                else:
                    nc.vector.tensor_copy(
                        kT_sb[:, ob * N + t0 : ob * N + t0 + 512], psv[:, :]
                    )

            def v_unit(t128, o0, w):
                """one v unit: 128 tokens x [o0, o0+w) v-dims, written
                (bf16) into the vaug slot layout"""
                tch, tb = divmod(t128, 4)
                psv = ps2.tile(
                    [128, 512], F32, tag="psV", bufs=2, name=f"v{t128}_{o0}"
                )
                for ci in range(CB):
                    nc.tensor.matmul(
                        psv[:, :w],
                        xts[tch][ci][:, tb * 128 : (tb + 1) * 128],
                        wq(ci, 2 * C + o0, w),
                        start=(ci == 0),
                        stop=(ci == CB - 1),
                    )
                nh = w // D
                src = psv[:, :w].rearrange("p (h x) -> p h x", x=D)
                h0 = o0 // D
                base = t128 * VW + h0 * (D + 1)
                dst = vaug_sb[:, base : base + nh * (D + 1)].rearrange(
                    "p (h x) -> p h x", x=D + 1
                )[:, :, :D]
                nc.vector.tensor_copy(dst, src)

            def proj_unit(ob, qc):
                """output projection for out-dims block ob, query half qc
                (contracts all 6 attnT pair-blocks), bias add + DMA out"""
                psp = ps2.tile(
                    [128, 512], F32, tag="psV", bufs=2, name=f"prj{ob}_{qc}"
                )
                for cb in range(CB):
                    nc.tensor.matmul(
                        psp[:, :],
                        wproj_sb[:, cb * C + ob * 128 : cb * C + (ob + 1) * 128],
                        attnT_sb[:, cb * NQ + qc * 512 : cb * NQ + (qc + 1) * 512],
                        start=(cb == 0),
                        stop=(cb == CB - 1),
                    )
                ot = op_pool.tile([128, 512], BF16, tag="out")
                nc.vector.tensor_scalar_add(
                    ot[:, :], psp[:, :], bias_sb[:, ob : ob + 1]
                )
                nc.sync.dma_start(
                    out=outT[ob * 128 : (ob + 1) * 128, qc * 512 : (qc + 1) * 512],
                    in_=ot[:, :],
                )

            # qc=1 projection: partial sums over pairs 0-4 ride as filler
            # in the last pass; only the pair-5 term + add/bias is tail
            projp_sb = [
                p23.tile([128, 512], F32, name=f"pp{ob}") for ob in range(CB)
            ]

            def proj_partial(ob):
                psp = ps2.tile(
                    [128, 512], F32, tag="psV", bufs=2, name=f"prjp{ob}"
                )
                for cb in range(CB - 1):
                    nc.tensor.matmul(
                        psp[:, :],
                        wproj_sb[:, cb * C + ob * 128 : cb * C + (ob + 1) * 128],
                        attnT_sb[:, cb * NQ + 512 : cb * NQ + 1024],
                        start=(cb == 0),
                        stop=(cb == CB - 2),
                    )
                nc.vector.tensor_copy(projp_sb[ob][:, :], psp[:, :])

            def proj_final(ob):
                cb = CB - 1
                psp = ps2.tile(
                    [128, 512], F32, tag="psV", bufs=2, name=f"prjf{ob}"
                )
                nc.tensor.matmul(
                    psp[:, :],
                    wproj_sb[:, cb * C + ob * 128 : cb * C + (ob + 1) * 128],
                    attnT_sb[:, cb * NQ + 512 : cb * NQ + 1024],
                    start=True,
                    stop=True,
                )
                ot = op_pool.tile([128, 512], BF16, tag="out")
                # (psp + bias) + partial, fused on DVE
                nc.vector.scalar_tensor_tensor(
                    ot[:, :],
                    psp[:, :],
                    bias_sb[:, ob : ob + 1],
                    projp_sb[ob][:, :],
                    op0=mybir.AluOpType.add,
                    op1=mybir.AluOpType.add,
                )
                nc.sync.dma_start(
                    out=outT[ob * 128 : (ob + 1) * 128, 512:1024],
                    in_=ot[:, :],
                )

            # ---- filler schedule --------------------------------------
            # filler[t] = list of closures to emit in step t's filler slot
            filler = [[] for _ in range(NSTEPS)]

            def _sched(t, fn):
                filler[t].append(fn)

            # remaining pair-0 k chunks first (their x chunks land before
            # the v weights): k tch needed by scores kb=4*tch (step 2*tch)
            for tch in range(1, 4):
                _sched(2 * (tch - 1), (lambda t=tch: kq_unit(0, t, False)))
            # v blocks just-in-time for pass 0: block kb in step kb//2
            for kb in range(TB):
                _sched(kb // 2, (lambda kb=kb: v_unit(kb, 0, 512)))
                _sched(kb // 2, (lambda kb=kb: v_unit(kb, 512, 256)))
            # pairs 1-5: k tch j needed by step 8*hb + 2*j, q tch0 by 8*hb
            for hb in range(1, HB):
                s0 = 8 * (hb - 1) + 3
                for j in range(4):
                    _sched(min(s0 + j, 8 * hb + 2 * j - 2),
                           (lambda h=hb, j=j: kq_unit(h, j, False)))
                _sched(8 * hb - 2, (lambda h=hb: kq_unit(h, 0, True)))
            # q tch1 units needed by pass (hb, 1) = step 48 + 8*hb
            for hb in range(HB):
                _sched(40 + hb, (lambda h=hb: kq_unit(h, 1, True)))
            # out-proj qc=0: attnT qc0 final after pass idx 5 epilogue
            # (multiply lands ~step 50); spread over the ACT-bound tail
            for ob in range(CB):
                _sched(54 + 4 * ob, (lambda o=ob: proj_unit(o, 0)))
            # out-proj qc=1 partials over pairs 0-4: pair-4 attnT qc1 is
            # multiplied at step 89 (pass idx 10 epilogue)
            for ob in range(CB):
                _sched(90 + ob, (lambda o=ob: proj_partial(o)))

            # ---- attention pipeline -----------------------------------
            def epi_pe(hb_, qc_, outs_):
                """PE part of a pass's normalization epilogue. The two
                heads' 1/denom broadcasts go to different column strips of
                one PSUM tile (col tiling) so they run concurrently."""
                psb = ps2.tile(
                    [128, 512], F32, tag="psV", bufs=2,
                    name=f"psb{hb_}_{qc_}",
                )
                for hh_ in range(2):
                    nc.tensor.matmul(
                        psb[64 * hh_ : 64 * hh_ + 64, :],
                        ones_sb[:, :],
                        outs_[hh_][1][:, :],
                        start=True,
                        stop=True,
                    )
                for hh_ in range(2):
                    nc.vector.tensor_mul(
                        attnT_sb[
                            64 * hh_ : 64 * hh_ + 64,
                            hb_ * NQ + qc_ * 512 : hb_ * NQ + (qc_ + 1) * 512,
                        ],
                        psb[64 * hh_ : 64 * hh_ + 64, :],
                        outs_[hh_][0][:, :],
                    )

            def emit_scores(hb, qc, kb2):
                """scores for k-blocks kb2, kb2+1 (both heads) -> psS
                tiles + exp -> probs; returns [(kb, probs), ...]"""
                q0 = hb * NQ + qc * 512
                out = []
                for kb in (kb2, kb2 + 1):
                    sc = ps2.tile(
                        [128, NQ], F32, tag="psS", bufs=2,
                        name=f"sc{hb}_{qc}_{kb}",
                    )
                    for hh in range(2):
                        p0 = 64 * hh
                        nc.tensor.matmul(
                            sc[:, hh * 512 : (hh + 1) * 512],
                            kT_sb[
                                p0 : p0 + 64,
                                hb * N + kb * 128 : hb * N + (kb + 1) * 128,
                            ],
                            qT_sb[p0 : p0 + 64, q0 : q0 + 512],
                            start=True,
                            stop=True,
                            tile_position=(p0, 0),
                        )
                    out.append((kb, sc))
                return out

            def emit_exps(pending_sc):
                out = []
                for kb, sc in pending_sc:
                    pb = hp.tile([128, NQ], BF16, tag="probs")
                    nc.scalar.activation(pb[:, :], sc[:, :], EXP, scale=SCALE)
                    out.append((kb, pb))
                return out

            def av_mms(accs, hb, pkb, ppb, heads=(0, 1)):
                for hh in heads:
                    vs = pkb * VW + (2 * hb + hh) * (D + 1)
                    nc.tensor.matmul(
                        accs[hh][0:65, :],
                        vaug_sb[:, vs : vs + D + 1],
                        ppb[:, hh * 512 : (hh + 1) * 512],
                        start=(pkb == 0),
                        stop=(pkb == TB - 1),
                    )

            def drain(accs):
                """epilogue: drain accumulators (on the otherwise-idle
                Pool engine) + 1/denominator (DVE)"""
                outs = []
                for hh in range(2):
                    acc = accs[hh]
                    cpy = mi.tile([64, 512], F32, tag="cpy")
                    nc.vector.tensor_copy(cpy[:, :], acc[0:64, :])
                    den = mi.tile([1, 512], F32, tag="den")
                    nc.vector.tensor_copy(den[:, :], acc[64:65, :])
                    rec = mi.tile([1, 512], F32, tag="rec")
                    nc.vector.reciprocal_approx_fast(rec[:, :], den[:, :])
                    row = mi.tile([1, 512], BF16, tag="row")
                    nc.vector.tensor_copy(row[:, :], rec[:, :])
                    outs.append((cpy, row))
                return outs

            # ---- pre-phase: minimal (pair-0 k/q for the first chunk) --
            kq_unit(0, 0, False)
            kq_unit(0, 0, True)

            # ---- the flat pipeline ------------------------------------
            accs = None  # current pass's psA accumulators
            prev_probs = []  # [(kb, probs)] awaiting attn@v
            prev_pass = None  # (hb, qc) owning prev_probs
            pend_epi = None  # (hb, qc, outs, due_step)

            def new_accs(hb, qc):
                return [
                    ps2.tile(
                        [128, 512], F32, tag="psA", bufs=2,
                        name=f"acc{hb}_{qc}_{i}",
                    )
                    for i in range(2)
                ]

            for t in range(NSTEPS):
                hb, qc = PASSES[t // 8]
                kb2 = (t % 8) * 2
                # mid-pass: scores first (keeps ACT fed); at boundaries the
                # new pass's scores wait on the exp pipeline, so run the
                # previous pass's ready attn@v first to keep the PE busy
                if kb2 == 0 and prev_probs:
                    for pkb, ppb in prev_probs:
                        av_mms(accs, prev_pass[0], pkb, ppb)
                    pend_sc = emit_scores(hb, qc, kb2)
                else:
                    pend_sc = emit_scores(hb, qc, kb2)
                    for pkb, ppb in prev_probs:
                        av_mms(accs, prev_pass[0], pkb, ppb)
                if prev_probs and prev_probs[-1][0] == TB - 1:
                    # previous pass complete: drain + defer its epilogue,
                    # then recycle the acc slots for the new pass
                    outs = drain(accs)
                    pend_epi = (prev_pass[0], prev_pass[1], outs, t + 1)
                    accs = new_accs(hb, qc)
                elif accs is None:
                    accs = new_accs(hb, qc)
                # filler work
                for fn in filler[t]:
                    fn()
                # deferred epilogue of the pass before last
                if pend_epi is not None and t >= pend_epi[3]:
                    epi_pe(pend_epi[0], pend_epi[1], pend_epi[2])
                    pend_epi = None
                # exp of this step's scores
                prev_probs = emit_exps(pend_sc)
                prev_pass = (hb, qc)

            # ---- tail: last pass's attn@v drain + epilogue ------------
            if pend_epi is not None:
                epi_pe(pend_epi[0], pend_epi[1], pend_epi[2])
                pend_epi = None
            for pkb, ppb in prev_probs:
                av_mms(accs, prev_pass[0], pkb, ppb)
            # drain on ACT (idle after the final exp) + DVE for recip
            outs = []
            for hh in range(2):
                acc = accs[hh]
                cpy = mi.tile([64, 512], F32, tag="cpy")
                nc.scalar.copy(cpy[:, :], acc[0:64, :])
                den = mi.tile([1, 512], F32, tag="den")
                nc.scalar.copy(den[:, :], acc[64:65, :])
                rec = mi.tile([1, 512], F32, tag="rec")
                nc.vector.reciprocal_approx_fast(rec[:, :], den[:, :])
                row = mi.tile([1, 512], BF16, tag="row")
                nc.vector.tensor_copy(row[:, :], rec[:, :])
                outs.append((cpy, row))
            epi_pe(prev_pass[0], prev_pass[1], outs)

            # ---- output projection, qc=1 final terms ------------------
            for ob in range(CB):
                proj_final(ob)

    nc.finalize()
    return nc


_NC_CACHE = []


def _get_nc():
    if not _NC_CACHE:
        _NC_CACHE.append(_build())
    return _NC_CACHE[0]


def kernel(x, w_qkv, w_proj, b_proj):
    x = np.asarray(x, dtype=np.float32)
    w_qkv = np.asarray(w_qkv, dtype=np.float32)
    w_proj = np.asarray(w_proj, dtype=np.float32)
    b_proj = np.asarray(b_proj, dtype=np.float32)

    nc = _get_nc()

    wqkvT = w_qkv.T.astype(ml_dtypes.bfloat16)  # [C, 3C]
    wq3 = np.ascontiguousarray(wqkvT).reshape(CB, 128, 3 * C)  # [ci, p, o]
    wqp = np.concatenate(
        [
            wq3[:, :, o0 : o0 + w].transpose(1, 0, 2).reshape(128, CB * w)
            for o0, w in _WQ_GROUPS
        ],
        axis=1,
    )
    wqp = np.ascontiguousarray(wqp)
    # SBUF images: wproj cols = [ci][o], bias cols = [ci]
    wprojp = np.ascontiguousarray(
        w_proj.T.astype(ml_dtypes.bfloat16).reshape(CB, 128, C)
        .transpose(1, 0, 2)
        .reshape(128, CB * C)
    )
    biasp = np.ascontiguousarray(
        b_proj.astype(np.float32).reshape(CB, 128).T
    )

    in_maps = []
    for core in range(N_CORES):
        b, half = divmod(core, 2)
        # own 1024 query tokens first, then the other half (key order
        # within attention is permutation-invariant)
        mine = x[b, half * NQ : (half + 1) * NQ].T
        other = x[b, (1 - half) * NQ : (2 - half) * NQ].T
        xTc = np.concatenate([mine, other], axis=1).astype(ml_dtypes.bfloat16)
        # pack to the SBUF image: cols = [tch][ci][t]
        xTp = np.ascontiguousarray(
            xTc.reshape(CB, 128, 4, 512).transpose(1, 2, 0, 3).reshape(128, CB * N)
        )
        in_maps.append({"xTp": xTp, "wqp": wqp, "wprojp": wprojp, "biasp": biasp})

    res = run_bass_kernel_spmd(nc, in_maps, core_ids=list(range(N_CORES)))

    out = np.empty((B, N, C), dtype=np.float32)
    for core in range(N_CORES):
        b, half = divmod(core, 2)
        out[b, half * NQ : (half + 1) * NQ, :] = (
            res.results[core]["outT"].astype(np.float32).T
        )
    return out
